# revision 45
# baseline (speedup 1.0000x reference)
"""Graphormer attention head (block-diagonal, 32 graphs x 128 nodes) on 8 trn2 cores.

Sharding: graphs (row blocks of 128) across cores, 4 graphs per core.
Each core gets its 512 rows of x / b / edge_encoding plus replicated
projection weights.  b/edge slices are column-rotated on the host by
-512*core so the diagonal block of every core lands at the same columns,
letting all 8 cores run one SPMD program.

Math per graph g (128 rows, full 4096 columns), matching the reference:
    scores = (QK^T*scale + b + e) in-block ; (b + e) * -1e6 off-block
    softmax over the full row, then in-block probs @ V.

Key structural optimization: the off-block columns only influence the
output through (a) the row max M = max(max_in, -1e6*min_off) and (b)
their softmax-denominator contribution.  Whenever the off-block side
dominates (M = -1e6*min_off > max_in + ~100), every in-block numerator
exp(s_in - M) underflows to exactly 0 in fp32, so the output row is
exactly 0 no matter the denominator; whenever the in-block side
dominates, every off-block term exp(-1e6*t - M) underflows to exactly 0
and contributes nothing.  Therefore
    denom = sum_in exp(s_in - M) + exp(-1e6*min_off - M)
reproduces the reference output in both regimes (the second term is the
largest off-block exp; the others are <= it and vanish whenever the
in-block sum doesn't).  This removes the full-row exp pass entirely:
the only full-row work left is min(b+e) per row, done as ONE fused
custom-DVE op per off-block range (out = -(b+e), accum = running max).
b/e ship as fp8_e4m3 (4x less HBM traffic than fp32); the off-block
values only feed the min, which tolerates fp8 quantization because of
the 1e6 amplification.  The in-block window columns ship separately in
fp16 and are added to the QK^T scores via identity-matmul accumulation
on the PE, directly in PSUM.

Pipeline: b/e stream per half-graph (8 x 0.5 MB fp8 DMAs) interleaved
with the packed-constant chunks, so the serial DVE reduce chain tracks
the DMA stream instead of waiting behind one big constants transfer.
Engine placement per graph: PE does QK^T + window adds; DVE does the
fused off-block reduce and the small per-row stat math; ScalarE does
the window exp (with sum accumulation), the single off-block exp term,
and the output PSUM->SBUF copy with the 1/denom scale folded in.
"""

import math
import os

import numpy as np

os.environ.setdefault("MYCRO_LOCAL_CACHE", "1")

N = 4096
DIN = 512
DQ = 512
NCORES = 8
RPC = N // NCORES          # rows per core = 512
GPC = 4                    # graphs per core
M = 128                    # graph size
IC = DIN // 128            # 4 input chunks
OC = DQ // 128             # 4 output chunks
HALF = N // 2              # 2048: b/e stream in half-row chunks
NEG = -1000000.0
FMAX = 3.0e38

# packed-constants column offsets (fp16, [128, CW]).
# The Q/K projections are folded into one bilinear form: with homogeneous
# x~ = [x | 1] and W~q = [Wq | bq], W~k = [Wk | bk],
#   scores = scale * (x W_q^T + bq)(x W_k^T + bk)^T = x~ A x~^T,
#   A = scale * W~q^T W~k  (513x513, host-precomputed from weights only).
# Device computes uT = A[:512,:]^T x^T (a Q-like projection with A as the
# weight; the A[512,d] row folds in as the per-partition copy bias), the
# u512 row = x A[:512,512] + A[512,512], and per graph
#   S_g = sum_dc uT[dc][:,win] x^T[dc][:,win] + u512[win] (x) ones.
AD = DIN + 1               # 513
OFF_XT = 0                 # IC chunks of [128, RPC]
OFF_A = OFF_XT + IC * RPC  # IC chunks of [128, AD] (rows cc*128.. of A)
OFF_AB = OFF_A + IC * AD   # [128, IC+1]: col dc = A[512, dc*128:+128]; col IC = A[512,512] @ p0
OFF_WIN = OFF_AB + IC + 1  # [128, GPC*2*M]: per-graph fp16 b/e windows
OFF_WV = OFF_WIN + GPC * 2 * M
OFF_ID = OFF_WV + IC * DQ  # [128, 128] identity
OFF_B3 = OFF_ID + 128      # bv @ p64 (V-bias rank-1 matmul)
OFF_ON3 = OFF_B3 + DQ      # ones @ p0 and p64
CW = OFF_ON3 + DQ
# three constant chunks: [x | A+AB+windows | wv+id+biases]
C_SPLITS = (OFF_A, OFF_WV)

_cache = {}


def _register_custom_dve():
    """Register the fused (Src0+Src1)*imm2 max-reduce custom DVE op."""
    if "dveop" in _cache:
        return _cache["dveop"]
    import concourse.dve_ops as dve_ops
    from concourse.dve_ops import DveOp, _SUB_OPCODE_FOR_NAME, CUSTOM_DVE_SPECS
    from concourse.dve_spec import Spec, Src0, Src1, C1, C2, maxx, lower, _has_src1
    from concourse.dve_uop import DveOpSpec
    from concourse.dve_table_gen import dve_ver_for

    name = "ADD_SCALE_MAXRED"

    def _ref(in0, in1, c0, c1, c2):
        b = ((in0.astype(np.float32) + in1.astype(np.float32)) * c2)
        b = b.astype(np.float32)
        acc = b.reshape(b.shape[0], -1).max(axis=-1, keepdims=True)
        return b, np.maximum(acc, c1)

    spec = Spec(body=(Src0 + Src1) * C2, accum=maxx, accum_init=C1,
                reference=_ref)
    ver = dve_ver_for("TRN2")
    if name in _SUB_OPCODE_FOR_NAME:
        op = next(o for o in dve_ops.OPS if o.name == name)
        _cache["dveop"] = op
        return op
    row = max(_SUB_OPCODE_FOR_NAME.values()) + 1
    tmp = DveOpSpec(name=name, opcode=row, uops=lower(spec, ver=ver),
                    rd1_en=_has_src1(spec))
    op = DveOp(name, spec, subdim=False, uops_sha={ver: tmp.sha(ver)})
    dve_ops.OPS.append(op)
    _SUB_OPCODE_FOR_NAME[name] = row
    CUSTOM_DVE_SPECS[name] = spec
    _cache["dveop"] = op
    return op


def _build_bass():
    import concourse.mybir as mybir
    import concourse.tile as tile
    from concourse import bacc

    addmax = _register_custom_dve()

    f32 = mybir.dt.float32
    f16 = mybir.dt.float16
    f8 = mybir.dt.float8e4
    Alu = mybir.AluOpType
    Act = mybir.ActivationFunctionType
    Axis = mybir.AxisListType

    nc = bacc.Bacc("TRN2", target_bir_lowering=False)

    # be_in[g] = [b | e] rows of graph g (fp8)
    be_in = nc.dram_tensor("be_in", [GPC, 128, 2 * N], f8,
                           kind="ExternalInput")
    consts = nc.dram_tensor("consts", [128, CW], f16, kind="ExternalInput")
    out = nc.dram_tensor("out", [RPC, DQ], f16, kind="ExternalOutput")

    with tile.TileContext(nc) as tc:
        with (
            tc.tile_pool(name="const", bufs=1) as const,
            tc.tile_pool(name="qkv", bufs=1) as qkv,
            tc.tile_pool(name="big", bufs=8) as big,
            tc.tile_pool(name="small", bufs=4) as small,
            tc.tile_pool(name="stat", bufs=20) as stat,
            tc.tile_pool(name="psA", bufs=2, space="PSUM") as psA,
            tc.tile_pool(name="psS", bufs=4, space="PSUM") as psS,
            tc.tile_pool(name="psT", bufs=1, space="PSUM") as psT,
            tc.tile_pool(name="psO", bufs=1, space="PSUM") as psO,
        ):
            # ---- DMA issue order: be halves stream continuously; constant
            # chunks and the tiny window tensor slot in between.
            c_t = const.tile([128, CW], f16, tag="consts")
            be_t = [big.tile([128, 2 * N], f8, tag="be", name=f"be{g}")
                    for g in range(GPC)]
            s0, s1 = C_SPLITS
            nc.sync.dma_start(out=c_t[:, 0:s0], in_=consts[:, 0:s0])
            nc.sync.dma_start(out=be_t[0][:], in_=be_in[0])
            nc.sync.dma_start(out=c_t[:, s0:s1], in_=consts[:, s0:s1])
            nc.sync.dma_start(out=be_t[1][:], in_=be_in[1])
            nc.sync.dma_start(out=c_t[:, s1:CW], in_=consts[:, s1:CW])
            nc.sync.dma_start(out=be_t[2][:], in_=be_in[2])
            nc.sync.dma_start(out=be_t[3][:], in_=be_in[3])

            def xT(ic):
                return c_t[:, OFF_XT + ic * RPC:OFF_XT + (ic + 1) * RPC]

            def wslice(off, ic, c0, c1):
                base = off + ic * DQ
                return c_t[:, base + c0:base + c1]

            id_t = c_t[:, OFF_ID:OFF_ID + 128]
            bv_r = c_t[64:65, OFF_B3:OFF_B3 + DQ]
            on0_r = c_t[0:1, OFF_ON3:OFF_ON3 + DQ]
            on64_r = c_t[64:65, OFF_ON3:OFF_ON3 + DQ]

            def aslice(ic, c0, c1):
                base = OFF_A + ic * AD
                return c_t[:, base + c0:base + c1]

            # ---- uT = A^T x~^T (Q-like projection; A[512,:] row folds in
            # as the per-partition ScalarE copy bias) and the u512 row.
            uT_t, v_t = [], []
            for dc in range(OC):
                ps = psA.tile([128, RPC], f32)
                for ic in range(IC):
                    nc.tensor.matmul(
                        ps[:], aslice(ic, dc * 128, (dc + 1) * 128),
                        xT(ic), start=(ic == 0), stop=(ic == IC - 1))
                t = qkv.tile([128, RPC], f16, tag=f"uT{dc}")
                nc.scalar.activation(t[:], ps[:], Act.Identity,
                                     bias=c_t[:, OFF_AB + dc:OFF_AB + dc + 1])
                uT_t.append(t)
            psu = psT.tile([1, RPC], f32, tag="tp", name="psu")
            for ic in range(IC):
                nc.tensor.matmul(psu[:], aslice(ic, DIN, AD), xT(ic),
                                 start=(ic == 0), stop=(ic == IC - 1))
            u512_t = qkv.tile([1, RPC], f16, tag="u512")
            nc.scalar.activation(u512_t[:], psu[:], Act.Identity,
                                 bias=c_t[0:1, OFF_AB + IC:OFF_AB + IC + 1])

            # ---- window scores for ALL graphs right after uT (so the DVE's
            # window-max reduces are never starved), then V projections.
            sps_t = []
            for g in range(GPC):
                w0 = g * M
                w1 = w0 + M
                sps = psS.tile([128, M], f32, tag="sps", name=f"sps{g}")
                for dc in range(OC):
                    nc.tensor.matmul(
                        sps[:], uT_t[dc][:, w0:w1], xT(dc)[:, w0:w1],
                        start=(dc == 0), stop=False)
                nc.tensor.matmul(sps[:], u512_t[:1, w0:w1], on0_r[:, :M],
                                 start=False, stop=False)
                wb = g * 2 * M
                nc.tensor.matmul(sps[:], id_t,
                                 c_t[:, OFF_WIN + wb:OFF_WIN + wb + M],
                                 start=False, stop=False)
                nc.tensor.matmul(sps[:], id_t,
                                 c_t[:, OFF_WIN + wb + M:OFF_WIN + wb + 2 * M],
                                 start=False, stop=True)
                sps_t.append(sps)


            # ---- off-block reduces + per-graph softmax tails.
            # The fused custom-DVE chain owns the DVE queue; each graph's
            # stat/tail ops are emitted a couple of graphs behind so they
            # are already data-ready when the in-order DVE reaches them
            # (no head-of-line blocking of the reduce chain).
            def customs(g):
                w0 = g * M
                w1 = w0 + M
                beh = be_t[g]
                mneg = stat.tile([128, 1], f32, name=f"mneg{g}")
                if w0 == 0:
                    nc.vector._custom_dve(
                        addmax, out=beh[:, M:N], in0=beh[:, M:N],
                        in1=beh[:, N + M:2 * N], s1=-FMAX, imm2=-1.0,
                        accum_out=mneg[:])
                else:
                    mn_0 = stat.tile([128, 1], f32, name=f"mn0{g}")
                    nc.vector._custom_dve(
                        addmax, out=beh[:, 0:w0], in0=beh[:, 0:w0],
                        in1=beh[:, N:N + w0], s1=-FMAX, imm2=-1.0,
                        accum_out=mn_0[:])
                    nc.vector._custom_dve(
                        addmax, out=beh[:, w1:N], in0=beh[:, w1:N],
                        in1=beh[:, N + w1:2 * N], s1=mn_0[:], imm2=-1.0,
                        accum_out=mneg[:])
                return mneg

            def tail_a(g, mneg):
                sps = sps_t[g]
                mxn = stat.tile([128, 1], f32, name=f"mxn{g}")
                nc.vector.tensor_reduce(mxn[:], sps[:], axis=Axis.X,
                                        op=Alu.max, negate=True)
                negM = stat.tile([128, 1], f32, name=f"negM{g}")
                nc.vector.tensor_scalar(negM[:], mneg[:], NEG, mxn[:],
                                        Alu.mult, Alu.min)
                p_t = small.tile([128, M], f16, tag="p")
                s_sum = stat.tile([128, 1], f32, name=f"ssum{g}")
                nc.scalar.activation(p_t[:], sps[:], Act.Exp,
                                     bias=negM[:], scale=1.0, accum_out=s_sum[:])
                d_off = stat.tile([128, 1], f32, name=f"doff{g}")
                nc.scalar.activation(d_off[:], mneg[:], Act.Exp,
                                     bias=negM[:], scale=-NEG)
                return p_t, s_sum, d_off

            def tail_b(g, st3):
                p_t, s_sum, d_off = st3
                denom = stat.tile([128, 1], f32, name=f"den{g}")
                nc.vector.scalar_tensor_tensor(denom[:], s_sum[:], 1.0, d_off[:],
                                               op0=Alu.mult, op1=Alu.add)
                rden = stat.tile([128, 1], f32, name=f"rden{g}")
                nc.vector.reciprocal(rden[:], denom[:])
                ptp = psT.tile([128, M], f16, tag="tp", name=f"ptp{g}")
                nc.tensor.transpose(ptp[:], p_t[:], id_t)
                pt_t = small.tile([128, M], f16, tag="pt")
                nc.scalar.copy(pt_t[:], ptp[:])
                ops = psO.tile([128, DQ], f32, tag="ops", name=f"ops{g}")
                nc.tensor.matmul(ops[:], pt_t[:], v_t[g][:], start=True, stop=True)
                o_t = small.tile([128, DQ], f16, tag="o")
                nc.scalar.activation(o_t[:], ops[:], Act.Copy,
                                     bias=0.0, scale=rden[:])
                nc.sync.dma_start(out=out[g * M:(g + 1) * M, :], in_=o_t[:])

            mnegs = [customs(g) for g in range(GPC)]
            sts = [tail_a(g, mnegs[g]) for g in range(GPC)]
            # V projections emitted after tail_a: PE runs them behind uT/sps,
            # the PSUM->SBUF copies ride the DVE after its reduce chain, and
            # ScalarE's exp/d_off ops are never queued behind V copies.
            for rc in range(GPC):
                ps = psA.tile([128, DQ], f32)
                for ic in range(IC):
                    nc.tensor.matmul(
                        ps[:], xT(ic)[:, rc * 128:(rc + 1) * 128],
                        wslice(OFF_WV, ic, 0, DQ), start=(ic == 0), stop=False)
                nc.tensor.matmul(ps[:], on64_r[:, :128], bv_r[:],
                                 start=False, stop=True)
                t = qkv.tile([128, DQ], f16, tag=f"v{rc}")
                nc.vector.tensor_copy(t[:], ps[:])
                v_t.append(t)
            for g in range(GPC):
                tail_b(g, sts[g])

    nc.compile()
    return nc


def _get_bass():
    if "nc" not in _cache:
        _cache["nc"] = _build_bass()
    return _cache["nc"]


def _prepare_in_maps(x, b, e, Wq, bq, Wk, bk, Wv, bv):
    import ml_dtypes

    f16 = np.float16
    f8 = ml_dtypes.float8_e4m3
    scale = 1.0 / math.sqrt(DQ)

    wvT = Wv.T.astype(f16)
    # bilinear form A = scale * [Wq|bq]^T [Wk|bk]  (weight-only precompute)
    wq_h = np.concatenate(
        [Wq.astype(np.float32), bq.astype(np.float32)[:, None]], axis=1)
    wk_h = np.concatenate(
        [Wk.astype(np.float32), bk.astype(np.float32)[:, None]], axis=1)
    A = (np.float32(scale) * (wq_h.T @ wk_h)).astype(np.float32)  # [AD, AD]

    in_maps = []
    for c in range(NCORES):
        rows = slice(c * RPC, (c + 1) * RPC)
        consts = np.zeros((128, CW), dtype=f16)
        xT_c = x[rows].astype(np.float32).T.astype(f16)   # [DIN, RPC]
        for ic in range(IC):
            rr = slice(ic * 128, (ic + 1) * 128)
            consts[:, OFF_XT + ic * RPC:OFF_XT + (ic + 1) * RPC] = xT_c[rr]
            consts[:, OFF_A + ic * AD:OFF_A + (ic + 1) * AD] = A[rr].astype(f16)
            consts[:, OFF_WV + ic * DQ:OFF_WV + (ic + 1) * DQ] = wvT[rr]
            consts[:, OFF_AB + ic] = A[DIN, ic * 128:(ic + 1) * 128].astype(f16)
        consts[0, OFF_AB + IC] = np.float16(A[DIN, DIN])
        consts[:, OFF_ID:OFF_ID + 128] = np.eye(128, dtype=f16)
        consts[64, OFF_B3:OFF_B3 + DQ] = bv.astype(np.float32).astype(f16)
        consts[0, OFF_ON3:OFF_ON3 + DQ] = 1.0
        consts[64, OFF_ON3:OFF_ON3 + DQ] = 1.0

        b_c = np.roll(b[rows], -c * RPC, axis=1)
        e_c = np.roll(e[rows], -c * RPC, axis=1)
        be = np.empty((GPC, 128, 2 * N), dtype=f8)
        for g in range(GPC):
            gr = slice(g * M, (g + 1) * M)
            be[g, :, :N] = b_c[gr].astype(f8)
            be[g, :, N:] = e_c[gr].astype(f8)
            wb = OFF_WIN + g * 2 * M
            consts[:, wb:wb + M] = b_c[gr, gr].astype(f16)
            consts[:, wb + M:wb + 2 * M] = e_c[gr, gr].astype(f16)

        in_maps.append({"be_in": be, "consts": consts})
    return in_maps


def _reference_numpy(x, b, e, ptr, Wq, bq, Wk, bk, Wv, bv):
    """Fallback for unexpected ptr layouts: straight fp32 numpy port."""
    n = x.shape[0]
    graph_id = np.searchsorted(ptr, np.arange(n), side="right") - 1
    mask = graph_id[:, None] == graph_id[None, :]
    q = x @ Wq.T + bq
    k = x @ Wk.T + bk
    v = x @ Wv.T + bv
    s = np.float32(1.0 / np.sqrt(np.float32(q.shape[-1])))
    a = np.where(mask, (q @ k.T) * s, np.float32(0.0))
    scores = (a + b + e) * np.where(mask, np.float32(1.0), np.float32(-1e6))
    m = scores.max(axis=-1, keepdims=True)
    ex = np.exp(scores - m, dtype=np.float32)
    soft = ex / ex.sum(axis=-1, keepdims=True)
    return ((soft * mask) @ v).astype(np.float32)


def _run(inputs, trace=False):
    from concourse.bass_utils import run_bass_kernel_spmd

    x = np.asarray(inputs["x"], dtype=np.float32)
    b = np.asarray(inputs["b"], dtype=np.float32)
    e = np.asarray(inputs["edge_encoding"], dtype=np.float32)
    ptr = np.asarray(inputs["ptr"])
    Wq = np.asarray(inputs["Wq"], dtype=np.float32)
    bq = np.asarray(inputs["bq"], dtype=np.float32)
    Wk = np.asarray(inputs["Wk"], dtype=np.float32)
    bk = np.asarray(inputs["bk"], dtype=np.float32)
    Wv = np.asarray(inputs["Wv"], dtype=np.float32)
    bv = np.asarray(inputs["bv"], dtype=np.float32)

    expected_ptr = np.arange(33, dtype=np.int64) * (N // 32)
    if (x.shape != (N, DIN) or ptr.shape != (33,)
            or not np.array_equal(ptr.astype(np.int64), expected_ptr)):
        return _reference_numpy(x, b, e, ptr, Wq, bq, Wk, bk, Wv, bv), None

    nc = _get_bass()
    in_maps = _prepare_in_maps(x, b, e, Wq, bq, Wk, bk, Wv, bv)
    res = run_bass_kernel_spmd(nc, in_maps, core_ids=list(range(NCORES)),
                               trace=trace)
    full = np.concatenate([res.results[c]["out"] for c in range(NCORES)], axis=0)
    return full.astype(np.float32), res


def kernel(**inputs):
    out, _ = _run(inputs, trace=False)
    return out


# revision 46
# speedup vs baseline: 1.0454x; 1.0454x over previous
"""Graphormer attention head (block-diagonal, 32 graphs x 128 nodes) on 8 trn2 cores.

Sharding: graphs (row blocks of 128) across cores, 4 graphs per core.
Each core gets its 512 rows of x / b / edge_encoding plus replicated
projection weights.  b/edge slices are column-rotated on the host by
-512*core so the diagonal block of every core lands at the same columns,
letting all 8 cores run one SPMD program.

Math per graph g (128 rows, full 4096 columns), matching the reference:
    scores = (QK^T*scale + b + e) in-block ; (b + e) * -1e6 off-block
    softmax over the full row, then in-block probs @ V.

Key structural optimization: the off-block columns only influence the
output through (a) the row max M = max(max_in, -1e6*min_off) and (b)
their softmax-denominator contribution.  Whenever the off-block side
dominates (M = -1e6*min_off > max_in + ~100), every in-block numerator
exp(s_in - M) underflows to exactly 0 in fp32, so the output row is
exactly 0 no matter the denominator; whenever the in-block side
dominates, every off-block term exp(-1e6*t - M) underflows to exactly 0
and contributes nothing.  Therefore
    denom = sum_in exp(s_in - M) + exp(-1e6*min_off - M)
reproduces the reference output in both regimes (the second term is the
largest off-block exp; the others are <= it and vanish whenever the
in-block sum doesn't).  This removes the full-row exp pass entirely:
the only full-row work left is min(b+e) per row, done as ONE fused
custom-DVE op per off-block range (out = -(b+e), accum = running max).
b/e ship as fp8_e4m3 (4x less HBM traffic than fp32); the off-block
values only feed the min, which tolerates fp8 quantization because of
the 1e6 amplification.  The in-block window columns ship separately in
fp16 and are added to the QK^T scores via identity-matmul accumulation
on the PE, directly in PSUM.

Pipeline: b/e stream per half-graph (8 x 0.5 MB fp8 DMAs) interleaved
with the packed-constant chunks, so the serial DVE reduce chain tracks
the DMA stream instead of waiting behind one big constants transfer.
Engine placement per graph: PE does QK^T + window adds; DVE does the
fused off-block reduce and the small per-row stat math; ScalarE does
the window exp (with sum accumulation), the single off-block exp term,
and the output PSUM->SBUF copy with the 1/denom scale folded in.
"""

import math
import os

import numpy as np

os.environ.setdefault("MYCRO_LOCAL_CACHE", "1")

N = 4096
DIN = 512
DQ = 512
NCORES = 8
RPC = N // NCORES          # rows per core = 512
GPC = 4                    # graphs per core
M = 128                    # graph size
IC = DIN // 128            # 4 input chunks
OC = DQ // 128             # 4 output chunks
HALF = N // 2              # 2048: b/e stream in half-row chunks
NEG = -1000000.0
FMAX = 3.0e38

# packed-constants column offsets (fp16, [128, CW]).
# The Q/K projections are folded into one bilinear form: with homogeneous
# x~ = [x | 1] and W~q = [Wq | bq], W~k = [Wk | bk],
#   scores = scale * (x W_q^T + bq)(x W_k^T + bk)^T = x~ A x~^T,
#   A = scale * W~q^T W~k  (513x513, host-precomputed from weights only).
# Device computes uT = A[:512,:]^T x^T (a Q-like projection with A as the
# weight; the A[512,d] row folds in as the per-partition copy bias), the
# u512 row = x A[:512,512] + A[512,512], and per graph
#   S_g = sum_dc uT[dc][:,win] x^T[dc][:,win] + u512[win] (x) ones.
AD = DIN + 1               # 513
OFF_XT = 0                 # IC chunks of [128, RPC]
OFF_A = OFF_XT + IC * RPC  # IC chunks of [128, AD] (rows cc*128.. of A)
OFF_AB = OFF_A + IC * AD   # [128, IC+1]: col dc = A[512, dc*128:+128]; col IC = A[512,512] @ p0
OFF_WIN = OFF_AB + IC + 1  # [128, GPC*2*M]: per-graph fp16 b/e windows
OFF_WV = OFF_WIN + GPC * 2 * M
OFF_ID = OFF_WV + IC * DQ  # [128, 128] identity
OFF_B3 = OFF_ID + 128      # bv @ p64 (V-bias rank-1 matmul)
OFF_ON3 = OFF_B3 + DQ      # ones @ p0 and p64
CW = OFF_ON3 + DQ
# three constant chunks: [x | A+AB+windows | wv+id+biases]
C_SPLITS = (OFF_A, OFF_WV)

_cache = {}


def _register_custom_dve():
    """Register the fused (Src0+Src1)*imm2 max-reduce custom DVE op."""
    if "dveop" in _cache:
        return _cache["dveop"]
    import concourse.dve_ops as dve_ops
    from concourse.dve_ops import DveOp, _SUB_OPCODE_FOR_NAME, CUSTOM_DVE_SPECS
    from concourse.dve_spec import Spec, Src0, Src1, C1, C2, maxx, lower, _has_src1
    from concourse.dve_uop import DveOpSpec
    from concourse.dve_table_gen import dve_ver_for

    name = "ADD_SCALE_MAXRED"

    def _ref(in0, in1, c0, c1, c2):
        b = ((in0.astype(np.float32) + in1.astype(np.float32)) * c2)
        b = b.astype(np.float32)
        acc = b.reshape(b.shape[0], -1).max(axis=-1, keepdims=True)
        return b, np.maximum(acc, c1)

    spec = Spec(body=(Src0 + Src1) * C2, accum=maxx, accum_init=C1,
                reference=_ref)
    ver = dve_ver_for("TRN2")
    if name in _SUB_OPCODE_FOR_NAME:
        op = next(o for o in dve_ops.OPS if o.name == name)
        _cache["dveop"] = op
        return op
    row = max(_SUB_OPCODE_FOR_NAME.values()) + 1
    tmp = DveOpSpec(name=name, opcode=row, uops=lower(spec, ver=ver),
                    rd1_en=_has_src1(spec))
    op = DveOp(name, spec, subdim=False, uops_sha={ver: tmp.sha(ver)})
    dve_ops.OPS.append(op)
    _SUB_OPCODE_FOR_NAME[name] = row
    CUSTOM_DVE_SPECS[name] = spec
    _cache["dveop"] = op
    return op


def _build_bass():
    import concourse.mybir as mybir
    import concourse.tile as tile
    from concourse import bacc

    addmax = _register_custom_dve()

    f32 = mybir.dt.float32
    f16 = mybir.dt.float16
    f8 = mybir.dt.float8e4
    Alu = mybir.AluOpType
    Act = mybir.ActivationFunctionType
    Axis = mybir.AxisListType

    nc = bacc.Bacc("TRN2", target_bir_lowering=False)

    # be_in[g] = [b | e] rows of graph g (fp8)
    be_in = nc.dram_tensor("be_in", [GPC, 128, 2 * N], f8,
                           kind="ExternalInput")
    consts = nc.dram_tensor("consts", [128, CW], f16, kind="ExternalInput")
    out = nc.dram_tensor("out", [RPC, DQ], f16, kind="ExternalOutput")

    with tile.TileContext(nc) as tc:
        with (
            tc.tile_pool(name="const", bufs=1) as const,
            tc.tile_pool(name="qkv", bufs=1) as qkv,
            tc.tile_pool(name="big", bufs=8) as big,
            tc.tile_pool(name="small", bufs=4) as small,
            tc.tile_pool(name="stat", bufs=20) as stat,
            tc.tile_pool(name="psA", bufs=2, space="PSUM") as psA,
            tc.tile_pool(name="psS", bufs=4, space="PSUM") as psS,
            tc.tile_pool(name="psT", bufs=1, space="PSUM") as psT,
            tc.tile_pool(name="psO", bufs=1, space="PSUM") as psO,
        ):
            # ---- DMA issue order: be halves stream continuously; constant
            # chunks and the tiny window tensor slot in between.
            c_t = const.tile([128, CW], f16, tag="consts")
            be_t = [big.tile([128, 2 * N], f8, tag="be", name=f"be{g}")
                    for g in range(GPC)]
            s0, s1 = C_SPLITS
            nc.sync.dma_start(out=c_t[:, 0:s0], in_=consts[:, 0:s0])
            nc.sync.dma_start(out=be_t[0][:], in_=be_in[0])
            nc.sync.dma_start(out=c_t[:, s0:s1], in_=consts[:, s0:s1])
            nc.sync.dma_start(out=be_t[1][:], in_=be_in[1])
            nc.sync.dma_start(out=c_t[:, s1:CW], in_=consts[:, s1:CW])
            nc.sync.dma_start(out=be_t[2][:], in_=be_in[2])
            nc.sync.dma_start(out=be_t[3][:], in_=be_in[3])

            def xT(ic):
                return c_t[:, OFF_XT + ic * RPC:OFF_XT + (ic + 1) * RPC]

            def wslice(off, ic, c0, c1):
                base = off + ic * DQ
                return c_t[:, base + c0:base + c1]

            id_t = c_t[:, OFF_ID:OFF_ID + 128]
            bv_r = c_t[64:65, OFF_B3:OFF_B3 + DQ]
            on0_r = c_t[0:1, OFF_ON3:OFF_ON3 + DQ]
            on64_r = c_t[64:65, OFF_ON3:OFF_ON3 + DQ]

            def aslice(ic, c0, c1):
                base = OFF_A + ic * AD
                return c_t[:, base + c0:base + c1]

            # ---- uT = A^T x~^T (Q-like projection; A[512,:] row folds in
            # as the per-partition ScalarE copy bias) and the u512 row.
            uT_t, v_t = [], []
            for dc in range(OC):
                ps = psA.tile([128, RPC], f32)
                for ic in range(IC):
                    nc.tensor.matmul(
                        ps[:], aslice(ic, dc * 128, (dc + 1) * 128),
                        xT(ic), start=(ic == 0), stop=(ic == IC - 1))
                t = qkv.tile([128, RPC], f16, tag=f"uT{dc}")
                nc.scalar.activation(t[:], ps[:], Act.Identity,
                                     bias=c_t[:, OFF_AB + dc:OFF_AB + dc + 1])
                uT_t.append(t)
            psu = psT.tile([1, RPC], f32, tag="tp", name="psu")
            for ic in range(IC):
                nc.tensor.matmul(psu[:], aslice(ic, DIN, AD), xT(ic),
                                 start=(ic == 0), stop=(ic == IC - 1))
            u512_t = qkv.tile([1, RPC], f16, tag="u512")
            nc.scalar.activation(u512_t[:], psu[:], Act.Identity,
                                 bias=c_t[0:1, OFF_AB + IC:OFF_AB + IC + 1])

            # ---- window scores for ALL graphs right after uT (so the DVE's
            # window-max reduces are never starved), then V projections.
            sps_t = []
            for g in range(GPC):
                w0 = g * M
                w1 = w0 + M
                sps = psS.tile([128, M], f32, tag="sps", name=f"sps{g}")
                for dc in range(OC):
                    nc.tensor.matmul(
                        sps[:], uT_t[dc][:, w0:w1], xT(dc)[:, w0:w1],
                        start=(dc == 0), stop=False)
                nc.tensor.matmul(sps[:], u512_t[:1, w0:w1], on0_r[:, :M],
                                 start=False, stop=False)
                wb = g * 2 * M
                nc.tensor.matmul(sps[:], id_t,
                                 c_t[:, OFF_WIN + wb:OFF_WIN + wb + M],
                                 start=False, stop=False)
                nc.tensor.matmul(sps[:], id_t,
                                 c_t[:, OFF_WIN + wb + M:OFF_WIN + wb + 2 * M],
                                 start=False, stop=True)
                sps_t.append(sps)

            for rc in range(GPC):
                ps = psA.tile([128, DQ], f32)
                for ic in range(IC):
                    nc.tensor.matmul(
                        ps[:], xT(ic)[:, rc * 128:(rc + 1) * 128],
                        wslice(OFF_WV, ic, 0, DQ), start=(ic == 0), stop=False)
                nc.tensor.matmul(ps[:], on64_r[:, :128], bv_r[:],
                                 start=False, stop=True)
                t = qkv.tile([128, DQ], f16, tag=f"v{rc}")
                nc.scalar.copy(t[:], ps[:])
                v_t.append(t)

            # ---- off-block reduces + per-graph softmax tails.
            # The fused custom-DVE chain owns the DVE queue; each graph's
            # stat/tail ops are emitted a couple of graphs behind so they
            # are already data-ready when the in-order DVE reaches them
            # (no head-of-line blocking of the reduce chain).
            def customs(g):
                w0 = g * M
                w1 = w0 + M
                beh = be_t[g]
                mneg = stat.tile([128, 1], f32, name=f"mneg{g}")
                if w0 == 0:
                    nc.vector._custom_dve(
                        addmax, out=beh[:, M:N], in0=beh[:, M:N],
                        in1=beh[:, N + M:2 * N], s1=-FMAX, imm2=-1.0,
                        accum_out=mneg[:])
                else:
                    mn_0 = stat.tile([128, 1], f32, name=f"mn0{g}")
                    nc.vector._custom_dve(
                        addmax, out=beh[:, 0:w0], in0=beh[:, 0:w0],
                        in1=beh[:, N:N + w0], s1=-FMAX, imm2=-1.0,
                        accum_out=mn_0[:])
                    nc.vector._custom_dve(
                        addmax, out=beh[:, w1:N], in0=beh[:, w1:N],
                        in1=beh[:, N + w1:2 * N], s1=mn_0[:], imm2=-1.0,
                        accum_out=mneg[:])
                return mneg

            def tail_a(g, mneg):
                sps = sps_t[g]
                mxn = stat.tile([128, 1], f32, name=f"mxn{g}")
                nc.vector.tensor_reduce(mxn[:], sps[:], axis=Axis.X,
                                        op=Alu.max, negate=True)
                negM = stat.tile([128, 1], f32, name=f"negM{g}")
                nc.vector.tensor_scalar(negM[:], mneg[:], NEG, mxn[:],
                                        Alu.mult, Alu.min)
                p_t = small.tile([128, M], f16, tag="p")
                s_sum = stat.tile([128, 1], f32, name=f"ssum{g}")
                nc.scalar.activation(p_t[:], sps[:], Act.Exp,
                                     bias=negM[:], scale=1.0, accum_out=s_sum[:])
                d_off = stat.tile([128, 1], f32, name=f"doff{g}")
                nc.scalar.activation(d_off[:], mneg[:], Act.Exp,
                                     bias=negM[:], scale=-NEG)
                return p_t, s_sum, d_off

            def tail_b(g, st3):
                p_t, s_sum, d_off = st3
                denom = stat.tile([128, 1], f32, name=f"den{g}")
                nc.vector.scalar_tensor_tensor(denom[:], s_sum[:], 1.0, d_off[:],
                                               op0=Alu.mult, op1=Alu.add)
                rden = stat.tile([128, 1], f32, name=f"rden{g}")
                nc.vector.reciprocal(rden[:], denom[:])
                ptp = psT.tile([128, M], f16, tag="tp", name=f"ptp{g}")
                nc.tensor.transpose(ptp[:], p_t[:], id_t)
                pt_t = small.tile([128, M], f16, tag="pt")
                nc.scalar.copy(pt_t[:], ptp[:])
                ops = psO.tile([128, DQ], f32, tag="ops", name=f"ops{g}")
                nc.tensor.matmul(ops[:], pt_t[:], v_t[g][:], start=True, stop=True)
                o_t = small.tile([128, DQ], f16, tag="o")
                nc.scalar.activation(o_t[:], ops[:], Act.Copy,
                                     bias=0.0, scale=rden[:])
                nc.sync.dma_start(out=out[g * M:(g + 1) * M, :], in_=o_t[:])

            mneg0 = customs(0)
            mneg1 = customs(1)
            mneg2 = customs(2)
            st0 = tail_a(0, mneg0)
            mneg3 = customs(3)
            st1 = tail_a(1, mneg1)
            tail_b(0, st0)
            st2 = tail_a(2, mneg2)
            tail_b(1, st1)
            st3 = tail_a(3, mneg3)
            tail_b(2, st2)
            tail_b(3, st3)

    nc.compile()
    return nc


def _get_bass():
    if "nc" not in _cache:
        _cache["nc"] = _build_bass()
    return _cache["nc"]


def _prepare_in_maps(x, b, e, Wq, bq, Wk, bk, Wv, bv):
    import ml_dtypes

    f16 = np.float16
    f8 = ml_dtypes.float8_e4m3
    scale = 1.0 / math.sqrt(DQ)

    wvT = Wv.T.astype(f16)
    # bilinear form A = scale * [Wq|bq]^T [Wk|bk]  (weight-only precompute)
    wq_h = np.concatenate(
        [Wq.astype(np.float32), bq.astype(np.float32)[:, None]], axis=1)
    wk_h = np.concatenate(
        [Wk.astype(np.float32), bk.astype(np.float32)[:, None]], axis=1)
    A = (np.float32(scale) * (wq_h.T @ wk_h)).astype(np.float32)  # [AD, AD]

    in_maps = []
    for c in range(NCORES):
        rows = slice(c * RPC, (c + 1) * RPC)
        consts = np.zeros((128, CW), dtype=f16)
        xT_c = x[rows].astype(np.float32).T.astype(f16)   # [DIN, RPC]
        for ic in range(IC):
            rr = slice(ic * 128, (ic + 1) * 128)
            consts[:, OFF_XT + ic * RPC:OFF_XT + (ic + 1) * RPC] = xT_c[rr]
            consts[:, OFF_A + ic * AD:OFF_A + (ic + 1) * AD] = A[rr].astype(f16)
            consts[:, OFF_WV + ic * DQ:OFF_WV + (ic + 1) * DQ] = wvT[rr]
            consts[:, OFF_AB + ic] = A[DIN, ic * 128:(ic + 1) * 128].astype(f16)
        consts[0, OFF_AB + IC] = np.float16(A[DIN, DIN])
        consts[:, OFF_ID:OFF_ID + 128] = np.eye(128, dtype=f16)
        consts[64, OFF_B3:OFF_B3 + DQ] = bv.astype(np.float32).astype(f16)
        consts[0, OFF_ON3:OFF_ON3 + DQ] = 1.0
        consts[64, OFF_ON3:OFF_ON3 + DQ] = 1.0

        b_c = np.roll(b[rows], -c * RPC, axis=1)
        e_c = np.roll(e[rows], -c * RPC, axis=1)
        be = np.empty((GPC, 128, 2 * N), dtype=f8)
        for g in range(GPC):
            gr = slice(g * M, (g + 1) * M)
            be[g, :, :N] = b_c[gr].astype(f8)
            be[g, :, N:] = e_c[gr].astype(f8)
            wb = OFF_WIN + g * 2 * M
            consts[:, wb:wb + M] = b_c[gr, gr].astype(f16)
            consts[:, wb + M:wb + 2 * M] = e_c[gr, gr].astype(f16)

        in_maps.append({"be_in": be, "consts": consts})
    return in_maps


def _reference_numpy(x, b, e, ptr, Wq, bq, Wk, bk, Wv, bv):
    """Fallback for unexpected ptr layouts: straight fp32 numpy port."""
    n = x.shape[0]
    graph_id = np.searchsorted(ptr, np.arange(n), side="right") - 1
    mask = graph_id[:, None] == graph_id[None, :]
    q = x @ Wq.T + bq
    k = x @ Wk.T + bk
    v = x @ Wv.T + bv
    s = np.float32(1.0 / np.sqrt(np.float32(q.shape[-1])))
    a = np.where(mask, (q @ k.T) * s, np.float32(0.0))
    scores = (a + b + e) * np.where(mask, np.float32(1.0), np.float32(-1e6))
    m = scores.max(axis=-1, keepdims=True)
    ex = np.exp(scores - m, dtype=np.float32)
    soft = ex / ex.sum(axis=-1, keepdims=True)
    return ((soft * mask) @ v).astype(np.float32)


def _run(inputs, trace=False):
    from concourse.bass_utils import run_bass_kernel_spmd

    x = np.asarray(inputs["x"], dtype=np.float32)
    b = np.asarray(inputs["b"], dtype=np.float32)
    e = np.asarray(inputs["edge_encoding"], dtype=np.float32)
    ptr = np.asarray(inputs["ptr"])
    Wq = np.asarray(inputs["Wq"], dtype=np.float32)
    bq = np.asarray(inputs["bq"], dtype=np.float32)
    Wk = np.asarray(inputs["Wk"], dtype=np.float32)
    bk = np.asarray(inputs["bk"], dtype=np.float32)
    Wv = np.asarray(inputs["Wv"], dtype=np.float32)
    bv = np.asarray(inputs["bv"], dtype=np.float32)

    expected_ptr = np.arange(33, dtype=np.int64) * (N // 32)
    if (x.shape != (N, DIN) or ptr.shape != (33,)
            or not np.array_equal(ptr.astype(np.int64), expected_ptr)):
        return _reference_numpy(x, b, e, ptr, Wq, bq, Wk, bk, Wv, bv), None

    nc = _get_bass()
    in_maps = _prepare_in_maps(x, b, e, Wq, bq, Wk, bk, Wv, bv)
    res = run_bass_kernel_spmd(nc, in_maps, core_ids=list(range(NCORES)),
                               trace=trace)
    full = np.concatenate([res.results[c]["out"] for c in range(NCORES)], axis=0)
    return full.astype(np.float32), res


def kernel(**inputs):
    out, _ = _run(inputs, trace=False)
    return out
